# revision 19
# baseline (speedup 1.0000x reference)
"""DGL-MPNN layer on 8 Trainium2 NeuronCores (edge-parallel sharding).

Math: W[e] = (ef[e] @ W_edge + b_edge).reshape(64,64)
      msg[e] = nf[src[e]] @ W[e];  agg = segment_sum(msg, dst); out = agg + nf + bias

Restructured as one dense matmul per edge block:
      z[e, 64*d+h] = ef_ext[e,d] * nf[src[e],h]   (ef_ext = [ef | 1],  d=0..16)
      msg = z @ W2ext            (W2ext[64d+h, o] = W_edge[d, 64h+o]; rows 1024+: b_edge)

Per core (6250 edges, sorted by dst, padded to 6272):
  - z^T chunks ([K=128, e] layout) are built on DVE by multiplying the
    host-transposed gather of node features (nfT, this core's input shard)
    with a host-replicated ef (broadcast along partitions).
  - msg^T accumulates in PSUM with W2 chunks stationary (moving dim 512).
  - msg^T -> msg via PE transposes.
  - segment-sum: edges sorted by dst, so each 128-node block's aggregate is
    a small sum of onehot(dst_local)^T @ msg_tile matmuls (onehot built by
    an is_equal tensor_scalar against an iota tile); blocks are written
    densely - no scatter DMA at all.
  - host sums the 8 partial aggregates + nf + bias.
"""

import os
import numpy as np
import ml_dtypes

N_NODES = 10000
N_EDGES = 50000
HID = 64
EDGE_DIM = 16
N_CORES = 8

E_PER = N_EDGES // N_CORES          # 6250
N_TILES = -(-E_PER // 128)          # 49
E_PAD = N_TILES * 128               # 6272
K_FULL = (EDGE_DIM + 1) * HID       # 1088 = 8 full chunks + 1 half chunk
N_CHUNKS = 9                        # chunks 0-7: K=128, chunk 8: K=64
EBLK = 512                          # msg^T moving-dim block
N_EBLK = -(-E_PAD // EBLK)          # 13 (last block 128 wide)
NODE_BLOCKS = -(-N_NODES // 128)    # 79

BF16 = ml_dtypes.bfloat16

_compiled = {}


def _build(inc_struct):
    """inc_struct: tuple over tiles of (first_block, last_block) per tile,
    flattened into one tuple (program structure: which (tile, block)
    incidences exist, grouped by block)."""
    import concourse.bacc as bacc
    import concourse.mybir as mybir
    import concourse.tile as tile

    # rebuild the (block -> [(tile, inc_idx)]) grouping from inc_struct
    incs = []  # (tile, block) in emission order, index = inc_idx
    by_block = {}
    idx = 0
    for t, (b0, b1) in enumerate(inc_struct):
        if b0 < 0:
            continue
        for b in range(b0, b1 + 1):
            by_block.setdefault(b, []).append((t, idx))
            incs.append((t, b))
            idx += 1
    n_inc = len(incs)

    nc = bacc.Bacc("TRN2", target_bir_lowering=False, debug=False,
                   num_devices=N_CORES)
    dt = mybir.dt

    nfT_in = nc.dram_tensor("nfT", [128, E_PAD], dt.bfloat16,
                            kind="ExternalInput").ap()
    efrep = nc.dram_tensor("efrep", [K_FULL, E_PAD], dt.bfloat16,
                           kind="ExternalInput").ap()
    w2 = nc.dram_tensor("w2", [N_CHUNKS * 128 * HID], dt.bfloat16,
                        kind="ExternalInput").ap()
    iota_in = nc.dram_tensor("iota", [128, 128], dt.bfloat16,
                             kind="ExternalInput").ap()
    ident_in = nc.dram_tensor("ident", [64, 64], dt.bfloat16,
                              kind="ExternalInput").ap()
    dloc_in = nc.dram_tensor("dloc", [128, max(n_inc, 1)], dt.float32,
                             kind="ExternalInput").ap()
    agg = nc.dram_tensor("agg", [N_NODES, HID], dt.float32,
                         kind="ExternalOutput").ap()

    with tile.TileContext(nc) as tc:
        with (
            tc.tile_pool(name="const", bufs=1) as cpool,
            tc.tile_pool(name="ef", bufs=2) as ef_pool,
            tc.tile_pool(name="zt", bufs=N_CHUNKS) as zt_pool,
            tc.tile_pool(name="oh", bufs=6) as oh_pool,
            tc.tile_pool(name="big", bufs=1) as big_pool,
        ):
            # --- constants / small inputs (ACT ring; ef uses sync ring) ---
            w2_sb = cpool.tile([128, N_CHUNKS, HID], dt.bfloat16)
            nc.scalar.dma_start(
                w2_sb[:], w2.rearrange("(c p o) -> p c o", c=N_CHUNKS, p=128))
            iota_sb = cpool.tile([128, 128], dt.bfloat16)
            nc.scalar.dma_start(iota_sb[:], iota_in[:])
            ident_sb = cpool.tile([64, 64], dt.bfloat16)
            nc.scalar.dma_start(ident_sb[:], ident_in[:])
            dloc_sb = cpool.tile([128, max(n_inc, 1)], dt.float32)
            nc.scalar.dma_start(dloc_sb[:], dloc_in[:])
            nfT = big_pool.tile([128, E_PAD], dt.bfloat16)
            nc.scalar.dma_start(nfT[:], nfT_in[:])

            msgT_sb = big_pool.tile([64, E_PAD], dt.bfloat16)
            msg_sb = big_pool.tile([128, N_TILES, HID], dt.bfloat16)

            # build all z^T chunks (resident; consumed by both passes)
            zts = []
            for c in range(N_CHUNKS):
                kp = 128 if c < 8 else 64
                ef_sb = ef_pool.tile([128, E_PAD], dt.bfloat16, tag="ef")
                nc.sync.dma_start(ef_sb[:kp, :],
                                  efrep[c * 128:c * 128 + kp, :])
                zt = zt_pool.tile([128, E_PAD], dt.bfloat16, tag="zt")
                nc.vector.tensor_tensor(
                    out=zt[:kp, :], in0=nfT[:kp, :], in1=ef_sb[:kp, :],
                    op=mybir.AluOpType.mult)
                zts.append(zt)

            # msg^T accumulation: two passes over e-blocks (PSUM is 8 banks)
            for blk_set in (range(0, 8), range(8, N_EBLK)):
                with tc.tile_pool(name="mm", bufs=8, space="PSUM") as ppool:
                    ptiles = {b: ppool.tile([64, EBLK], dt.float32,
                                            tag="mmp", name=f"mmp{b}")
                              for b in blk_set}
                    for c in range(N_CHUNKS):
                        kp = 128 if c < 8 else 64
                        for b in blk_set:
                            bw = min(EBLK, E_PAD - b * EBLK)
                            nc.tensor.matmul(
                                out=ptiles[b][:, :bw],
                                lhsT=w2_sb[:kp, c, :],
                                rhs=zts[c][:kp, b * EBLK:b * EBLK + bw],
                                start=(c == 0),
                                stop=(c == N_CHUNKS - 1),
                            )
                    for b in blk_set:
                        bw = min(EBLK, E_PAD - b * EBLK)
                        nc.scalar.copy(
                            out=msgT_sb[:, b * EBLK:b * EBLK + bw],
                            in_=ptiles[b][:, :bw])

            with (
                tc.tile_pool(name="trp", bufs=3, space="PSUM") as trp,
                tc.tile_pool(name="aggp", bufs=5, space="PSUM") as aggp,
            ):
                # transpose msg^T -> msg tiles [128e, 64]
                for t in range(N_TILES):
                    trt = trp.tile([128, HID], dt.bfloat16, tag="tr")
                    nc.tensor.transpose(
                        out=trt[:],
                        in_=msgT_sb[:, t * 128:(t + 1) * 128],
                        identity=ident_sb[:])
                    if t % 2 == 0:
                        nc.vector.tensor_copy(out=msg_sb[:, t, :], in_=trt[:])
                    else:
                        nc.scalar.copy(out=msg_sb[:, t, :], in_=trt[:])

                # segment-sum via onehot matmuls, densely per 128-node block
                for bi, b in enumerate(sorted(by_block)):
                    members = by_block[b]
                    ap = aggp.tile([128, HID], dt.float32, tag="aggps")
                    for j, (t, inc_idx) in enumerate(members):
                        oh = oh_pool.tile([128, 128], dt.bfloat16, tag="oh")
                        eng = nc.vector if inc_idx % 2 == 0 else nc.gpsimd
                        eng.tensor_scalar(
                            out=oh[:], in0=iota_sb[:],
                            scalar1=dloc_sb[:, inc_idx:inc_idx + 1],
                            scalar2=None,
                            op0=mybir.AluOpType.is_equal)
                        nc.tensor.matmul(
                            out=ap[:],
                            lhsT=oh[:],
                            rhs=msg_sb[:, t, :],
                            start=(j == 0),
                            stop=(j == len(members) - 1),
                        )
                    stage = oh_pool.tile([128, HID], dt.float32, tag="stage")
                    if bi % 2 == 0:
                        nc.vector.tensor_copy(out=stage[:], in_=ap[:])
                    else:
                        nc.scalar.copy(out=stage[:], in_=ap[:])
                    nrows = min(128, N_NODES - b * 128)
                    nc.sync.dma_start(agg[b * 128:b * 128 + nrows, :],
                                      stage[:nrows, :])

    nc.compile()
    return nc


def _get_compiled(inc_struct):
    if inc_struct not in _compiled:
        _compiled[inc_struct] = _build(inc_struct)
    return _compiled[inc_struct]


def kernel(nf, initial_ef, src, dst, W_edge, b_edge, bias):
    from concourse.bass_utils import run_bass_kernel_spmd

    nf = np.asarray(nf, dtype=np.float32)
    initial_ef = np.asarray(initial_ef, dtype=np.float32)
    src = np.asarray(src, dtype=np.int32)
    dst = np.asarray(dst, dtype=np.int32)
    W_edge = np.asarray(W_edge, dtype=np.float32)
    b_edge = np.asarray(b_edge, dtype=np.float32)
    bias = np.asarray(bias, dtype=np.float32)

    # ---- host-side shared prep ----
    nf_dup = np.concatenate([nf, nf], axis=1).astype(BF16)  # [N, 128]

    w2ext = np.empty((K_FULL, HID), dtype=np.float32)
    w2ext[:EDGE_DIM * HID] = (
        W_edge.reshape(EDGE_DIM, HID, HID).reshape(EDGE_DIM * HID, HID))
    w2ext[EDGE_DIM * HID:] = b_edge.reshape(HID, HID)
    w2_pad = np.zeros((N_CHUNKS * 128, HID), dtype=np.float32)
    w2_pad[:K_FULL] = w2ext
    w2_flat = w2_pad.astype(BF16).reshape(-1)

    ef_ext = np.empty((EDGE_DIM + 1, N_EDGES), dtype=np.float32)
    ef_ext[:EDGE_DIM] = initial_ef.T
    ef_ext[EDGE_DIM] = 1.0

    iota = np.broadcast_to(np.arange(128, dtype=np.float32), (128, 128))
    iota = np.ascontiguousarray(iota).astype(BF16)
    ident = np.eye(64, dtype=np.float32).astype(BF16)

    in_maps = []
    structs = []
    for k in range(N_CORES):
        e0, e1 = k * E_PER, (k + 1) * E_PER
        dst_k = dst[e0:e1]
        perm = np.argsort(dst_k, kind="stable")
        dst_s = dst_k[perm]
        src_s = src[e0:e1][perm]

        # nfT: host-side transposed gather of this core's edge-aligned
        # node features (the core's input shard)
        nfT = np.zeros((128, E_PAD), dtype=BF16)
        nfT[:, :E_PER] = nf_dup[src_s].T

        ef_k = np.zeros((EDGE_DIM + 1, E_PAD), dtype=np.float32)
        ef_k[:, :E_PER] = ef_ext[:, e0:e1][:, perm]
        efrep_k = np.repeat(ef_k.astype(BF16), HID, axis=0)  # [1088, E_PAD]

        # incidence structure: per tile, node-block range of its real edges
        inc_struct = []
        dloc_cols = []
        for t in range(N_TILES):
            lo, hi = t * 128, min((t + 1) * 128, E_PER)
            if lo >= E_PER:
                inc_struct.append((-1, -1))
                continue
            dtile = dst_s[lo:hi]
            b0, b1 = int(dtile[0]) // 128, int(dtile[-1]) // 128
            inc_struct.append((b0, b1))
            dpad = np.full(128, -1000.0, dtype=np.float32)
            dpad[:hi - lo] = dtile
            for b in range(b0, b1 + 1):
                dloc_cols.append(dpad - 128.0 * b)
        # order columns by (block, tile) to match _build's grouping
        order = []
        pos = 0
        tmp = []
        for t, (b0, b1) in enumerate(inc_struct):
            if b0 < 0:
                continue
            for b in range(b0, b1 + 1):
                tmp.append((b, t, pos))
                pos += 1
        n_inc = len(tmp)
        dloc = np.zeros((128, max(n_inc, 1)), dtype=np.float32)
        # _build enumerates incidences tile-major for inc_idx, then groups
        # by block; dloc column index must equal inc_idx (tile-major).
        for j, col in enumerate(dloc_cols):
            dloc[:, j] = col
        structs.append(tuple(inc_struct))

        in_maps.append({
            "nfT": nfT,
            "efrep": efrep_k,
            "w2": w2_flat,
            "iota": iota,
            "ident": ident,
            "dloc": dloc,
        })

    # all cores must share one program (SPMD): merge structures by taking
    # the union per tile (extra incidences just add zero contributions,
    # since out-of-block dst_local never equals iota)
    merged = []
    for t in range(N_TILES):
        b0s = [s[t][0] for s in structs if s[t][0] >= 0]
        b1s = [s[t][1] for s in structs if s[t][0] >= 0]
        if not b0s:
            merged.append((-1, -1))
        else:
            merged.append((min(b0s), max(b1s)))
    merged = tuple(merged)

    # re-derive per-core dloc for the merged structure
    for k in range(N_CORES):
        e0 = k * E_PER
        dst_k = dst[e0:e0 + E_PER]
        perm = np.argsort(dst_k, kind="stable")
        dst_s = dst_k[perm]
        cols = []
        for t, (b0, b1) in enumerate(merged):
            if b0 < 0:
                continue
            lo, hi = t * 128, min((t + 1) * 128, E_PER)
            dpad = np.full(128, -100000.0, dtype=np.float32)
            if lo < E_PER:
                dpad[:hi - lo] = dst_s[lo:hi]
            for b in range(b0, b1 + 1):
                cols.append(dpad - 128.0 * b)
        n_inc = len(cols)
        dloc = np.zeros((128, max(n_inc, 1)), dtype=np.float32)
        for j, col in enumerate(cols):
            dloc[:, j] = col
        in_maps[k]["dloc"] = dloc

    nc = _get_compiled(merged)
    res = run_bass_kernel_spmd(nc, in_maps, list(range(N_CORES)))

    partial = np.zeros((N_NODES, HID), dtype=np.float32)
    for k in range(N_CORES):
        partial += res.results[k]["agg"]
    return partial + nf + bias
